# revision 4
# baseline (speedup 1.0000x reference)
"""ArcFace loss kernel for 8 TRN2 NeuronCores (vocab/tensor-parallel).

reference:
    xn = normalize(x)               # [B, D]
    wn = normalize(weight)          # [C, D]
    logits = 64 * xn @ wn.T         # [B, C]
    loss = mean(CE(logits, label))

Strategy: shard classes C=100000 over 8 cores (12500 each, zero-padded to
12544 = 24*512 + 256). Host prepares normalized fp8(e4m3) operands scaled
by G=8 (so device cosines are 64*cos), pre-packed in the exact SBUF tile
layout so every weight-group DMA is 8KB-contiguous per partition. Each core
computes its logit shard with TensorE fp8 DoubleRow matmuls (K=256/op) into
fp32 PSUM.

The sum-of-exp over each PSUM tile is split across two engines so the PE
stream is the only bottleneck:
  - ScalarE: one fused exp(l - SHIFT) activation over psum banks 0..2
    (1536 cols) with row-accumulate ((1536+352)/1.2 + 182 ns = ~1.75us
    < PE tile time ~1.86us).
  - VectorE: Schraudolph exp on bank 3 (512 cols): i32 = rint(A*l + Beff)
    computed by one tensor_scalar (fp32 mul-add, round-to-nearest, HW
    verified), whose int32 bits ARE the fp32 exp image; a tensor_reduce
    over the bitcast sums it (~1.4us < PE).
  The Schraudolph mean-centering constant absorbs the (1+f)2^-f sawtooth
  (E[g]=1.040684); residual loss error ~5e-5 relative.

All weight DMAs ride the single sync HWDGE ring in exact consumption
order (the ring sprays all 16 SDMA engines at ~420GB/s, so a second ring
adds no bandwidth and only distorts arrival order): x, w0 k-halves (for
the earliest possible PE start), w1, w2, wtail, w3, w4, w5. Tail tiles are
scheduled mid-stream where ScalarE has slack, with one kept last so the
final activation is short. Each core returns per-row partial sums [128, 4];
the host gathers the 8 cores, removes the exact zero-pad contribution, and
finishes loss = mean(log Z + SHIFT - 64*cos_label) with host-exact label
cosines.
"""

import math
import numpy as np

import concourse.mybir as mybir
import concourse.tile as tile
from concourse import bacc
from concourse.bass_utils import run_bass_kernel_spmd

# Problem constants (hardcoded per harness contract).
B = 512
D = 512
C = 100000
S = 64.0
SHIFT = 20.0  # logsumexp shift; keeps Z ~1e-2 (HW Ln saturates below ~1e-19)
EPS = 1e-12
G = 8.0      # fp8 pre-scale on both operands: device cos' = G^2 * cos
NCORES = 8
CS = C // NCORES        # true classes per core = 12500
CHUNK = 512             # matmul moving free dim = one full PSUM bank
TAILC = 256             # tail chunk width (212 real + 44 pad cols)
CS_PAD = 24 * CHUNK + TAILC  # padded classes per core = 12544
GROUP = 4               # psum banks (512-col chunks) per group
NG_FULL = 6             # full groups of 4 chunks; + 1 tail group of 1 chunk
GCOLS = GROUP * CHUNK   # 2048 logit columns per full group
PB = 128                # partitions
KSUB = D // PB          # 4 contraction subtiles of 128
BBLK = B // PB          # 4 batch blocks
NG = NG_FULL + 1        # total groups per core
N_WARM = 24             # PE warm-up matmuls issued while the first DMAs land

ACT_COLS = 1472         # cols handled by ScalarE exp (1669ns/tile w/ accum)
DVE_COLS = GCOLS - ACT_COLS   # 576 cols on VectorE Schraudolph (1584ns/tile)

# Schraudolph exp-by-bitcast constants (fp32): exp(x) ~= bitcast_f32(
#   rint(A*x + B0)), mean-centered over the mantissa sawtooth.
SCH_A = float(2.0**23 / math.log(2.0))                   # 12102203.1616
SCH_EG = 1.040684490502804                               # E[(1+f)2^-f]
SCH_B0 = (127.0 - math.log(SCH_EG) / math.log(2.0)) * 2.0**23
SCH_BEFF = SCH_B0 - SCH_A * SHIFT                        # folds the shift

F32 = mybir.dt.float32
I32 = mybir.dt.int32
BF16 = mybir.dt.bfloat16
FP8 = mybir.dt.float8e4
NP_FP8 = mybir.dt.np(FP8)

# separate per-engine accumulator tiles (a shared tile creates a false
# cross-engine WAW serialization in Tile's tracker that stalls the PE)


def build_nc(ncores: int = NCORES):
    """Build the SPMD Bass graph."""
    nc = bacc.Bacc(
        "TRN2",
        target_bir_lowering=False,
        debug=False,
        num_devices=ncores,
    )

    # host-packed operands: per-partition-contiguous SBUF layouts
    wmain_ext = nc.dram_tensor(
        "wmain", [NG_FULL * PB, KSUB, GCOLS], FP8, kind="ExternalInput"
    )
    wtail_ext = nc.dram_tensor("wtail", [PB, KSUB, TAILC], FP8, kind="ExternalInput")
    xnt_ext = nc.dram_tensor("xnt", [PB, KSUB, B], FP8, kind="ExternalInput")
    zp_ext = nc.dram_tensor("zp", [PB, BBLK], F32, kind="ExternalOutput")

    with tile.TileContext(nc) as tc:
        with (
            tc.tile_pool(name="const", bufs=1) as cpool,
            tc.tile_pool(name="wpool", bufs=NG_FULL) as wpool,
            tc.tile_pool(name="dpool", bufs=3) as dpool,
            tc.tile_pool(name="ipool", bufs=3) as ipool,
        ):
            # ALL loads on the sync HWDGE ring, in consumption order. One
            # ring already sprays all 16 SDMA engines; order == arrival.
            xsb = cpool.tile([PB, KSUB, B], FP8)
            nc.sync.dma_start(out=xsb, in_=xnt_ext[:])

            wts = []
            for g in range(NG_FULL):
                wt = wpool.tile([PB, KSUB, GCOLS], FP8, name="wt", tag="w")
                wts.append(wt)
            wtail = cpool.tile([PB, KSUB, TAILC], FP8)
            # group 0 split in k-halves so the first matmuls can start
            # ~1.2us earlier (each half is 4KB/partition contiguous)
            nc.sync.dma_start(out=wts[0][:, 0:2, :], in_=wmain_ext[0:PB, 0:2, :])
            nc.sync.dma_start(out=wts[0][:, 2:4, :], in_=wmain_ext[0:PB, 2:4, :])
            nc.sync.dma_start(out=wts[1], in_=wmain_ext[PB : 2 * PB, :, :])
            nc.sync.dma_start(out=wts[2], in_=wmain_ext[2 * PB : 3 * PB, :, :])
            nc.sync.dma_start(out=wtail, in_=wtail_ext[:])
            nc.sync.dma_start(out=wts[3], in_=wmain_ext[3 * PB : 4 * PB, :, :])
            nc.sync.dma_start(out=wts[4], in_=wmain_ext[4 * PB : 5 * PB, :, :])
            nc.sync.dma_start(out=wts[5], in_=wmain_ext[5 * PB : 6 * PB, :, :])
            wts.append(wtail)

            # warm-up operand first so the PE can start immediately
            warm = cpool.tile([PB, 256], BF16)
            nc.vector.memset(warm, 0.0)

            # constants
            negs = cpool.tile([PB, 1], F32)
            nc.vector.memset(negs, -SHIFT)

            # per-row partial sums, one tile per engine
            pSa = cpool.tile([PB, BBLK, NG_FULL], F32)      # ScalarE accum
            pSv = cpool.tile([PB, BBLK, NG], F32)           # VectorE sums

            # preload the Exp activation table off the critical path
            dumdum = cpool.tile([PB, 1], BF16)
            nc.scalar.activation(
                out=dumdum, in_=negs,
                func=mybir.ActivationFunctionType.Exp, bias=negs, scale=1.0,
            )

            with tc.tile_pool(name="psmain", bufs=2, space="PSUM") as pspool:
                # PE warm-up: dependency-free matmuls so the HAM clock gate is
                # released by the time the first weight tiles arrive.
                ones_bf = nc.const_aps.aps[(BF16, 1.0)]
                warm_ps = pspool.tile(
                    [PB, GROUP, CHUNK], F32, name="warm_ps", tag="ps",
                )
                for _ in range(N_WARM):
                    nc.tensor.matmul(
                        out=warm_ps[0:1, 0, :256], lhsT=ones_bf, rhs=warm,
                        start=True, stop=True,
                    )

                # full groups in DMA-arrival order; tail tiles slotted where
                # ScalarE has slack, one kept last so the final activation
                # is short
                tiles = [(g, bb) for g in range(2) for bb in range(BBLK)]
                tiles += [(NG - 1, 0)]
                tiles += [(2, bb) for bb in range(BBLK)]
                tiles += [(NG - 1, 1)]
                tiles += [(3, bb) for bb in range(BBLK)]
                tiles += [(NG - 1, 2)]
                tiles += [(g, bb) for g in range(4, 6) for bb in range(BBLK)]
                tiles += [(NG - 1, 3)]
                for g, bb in tiles:
                    nsub = GROUP if g < NG_FULL else 1
                    wt = wts[g]
                    ps = pspool.tile(
                        [PB, nsub, CHUNK], F32, name="ps", tag="ps",
                        padded_shape=[PB, GROUP, CHUNK],
                    )
                    cw = CHUNK if g < NG_FULL else TAILC
                    for k2 in range(KSUB // 2):
                        for sub in range(nsub):
                            nc.tensor.matmul(
                                out=ps[:, sub : sub + 1, 0:cw],
                                lhsT=xsb[
                                    :, 2 * k2 : 2 * k2 + 2,
                                    bb * PB : (bb + 1) * PB,
                                ],
                                rhs=wt[
                                    :, 2 * k2 : 2 * k2 + 2,
                                    sub * cw : (sub + 1) * cw,
                                ],
                                start=(k2 == 0),
                                stop=(k2 == KSUB // 2 - 1),
                                perf_mode=mybir.MatmulPerfMode.DoubleRow,
                            )
                    flat = ps.rearrange("p s c -> p (s c)")
                    if g < NG_FULL:
                        # ScalarE: exp(l - SHIFT) over the first 1472 cols
                        dump = dpool.tile(
                            [PB, ACT_COLS], BF16, name="dump", tag="dump",
                        )
                        nc.scalar.activation(
                            out=dump,
                            in_=flat[:, 0:ACT_COLS],
                            func=mybir.ActivationFunctionType.Exp,
                            bias=negs,
                            scale=1.0,
                            accum_out=pSa[:, bb, g : g + 1],
                        )
                        # VectorE: Schraudolph exp over the rest, summed via
                        # the int32->fp32 bitcast
                        idump = ipool.tile(
                            [PB, DVE_COLS], I32, name="idump", tag="idump",
                        )
                        nc.vector.tensor_scalar(
                            out=idump,
                            in0=flat[:, ACT_COLS:GCOLS],
                            scalar1=SCH_A,
                            scalar2=SCH_BEFF,
                            op0=mybir.AluOpType.mult,
                            op1=mybir.AluOpType.add,
                        )
                        nc.vector.tensor_reduce(
                            pSv[:, bb, g : g + 1],
                            idump.bitcast(F32),
                            axis=mybir.AxisListType.X,
                            op=mybir.AluOpType.add,
                        )
                    else:
                        # tail tile: entirely VectorE (ScalarE has no slack;
                        # the 44 pad cols' Schraudolph exp(-SHIFT) image is
                        # bit-deterministic and removed on host)
                        idump = ipool.tile(
                            [PB, TAILC], I32, name="idumpt", tag="idump",
                            padded_shape=[PB, DVE_COLS],
                        )
                        nc.vector.tensor_scalar(
                            out=idump,
                            in0=ps[:, 0, 0:TAILC],
                            scalar1=SCH_A,
                            scalar2=SCH_BEFF,
                            op0=mybir.AluOpType.mult,
                            op1=mybir.AluOpType.add,
                        )
                        nc.vector.tensor_reduce(
                            pSv[:, bb, NG_FULL : NG_FULL + 1],
                            idump.bitcast(F32),
                            axis=mybir.AxisListType.X,
                            op=mybir.AluOpType.add,
                        )

            # Z partial per row: [128, BBLK] -> output (host sums the 8 cores)
            zpa = cpool.tile([PB, BBLK], F32)
            nc.vector.tensor_reduce(
                zpa, pSa, axis=mybir.AxisListType.X, op=mybir.AluOpType.add,
            )
            zpv = cpool.tile([PB, BBLK], F32)
            nc.vector.tensor_reduce(
                zpv, pSv, axis=mybir.AxisListType.X, op=mybir.AluOpType.add,
            )
            zp = cpool.tile([PB, BBLK], F32)
            nc.vector.tensor_add(out=zp, in0=zpa, in1=zpv)
            nc.sync.dma_start(out=zp_ext[:], in_=zp)

    nc.finalize()
    return nc


def prepare_inputs(x, weight, label, ncores: int = NCORES):
    """Host-side prep: normalize, G-scale, cast fp8, pack to SBUF layouts.

    Returns (in_maps, lc2) where lc2[p, j] = SHIFT - S*cos(x_b, w_label_b)
    for b = j*128 + p."""
    x = np.asarray(x, dtype=np.float32)
    weight = np.asarray(weight, dtype=np.float32)
    label = np.asarray(label).astype(np.int64)

    xn = x / np.maximum(
        np.sqrt(np.einsum("bd,bd->b", x, x, dtype=np.float64))[:, None], EPS
    ).astype(np.float32)
    wnorm = np.sqrt(np.einsum("cd,cd->c", weight, weight, dtype=np.float64))
    wn = weight / np.maximum(wnorm[:, None], EPS).astype(np.float32)

    # label cosine computed on host in f64 (exact vs fp32 reference)
    wl = wn[label]  # [B, D]
    label_cos = np.einsum("bd,bd->b", xn.astype(np.float64), wl.astype(np.float64))
    lc2 = (SHIFT - S * label_cos).astype(np.float64)  # [B]
    lc2_pj = np.ascontiguousarray(lc2.reshape(BBLK, PB).T)  # [128, BBLK]

    x8 = (G * xn).astype(NP_FP8)          # [B, D]
    w8 = (G * wn).astype(NP_FP8)          # [C, D]
    # xnt[p, ks, b] = x8[b, ks*128 + p]
    xp = np.ascontiguousarray(x8.reshape(B, KSUB, PB).transpose(2, 1, 0))

    in_maps = []
    for i in range(ncores):
        wp = np.zeros((CS_PAD, D), dtype=NP_FP8)
        wp[:CS] = w8[i * CS : (i + 1) * CS]
        # wmain[g*128+p, ks, col] = wp[g*2048 + col, ks*128 + p]
        main = (
            wp[: NG_FULL * GCOLS]
            .reshape(NG_FULL, GCOLS, KSUB, PB)
            .transpose(0, 3, 2, 1)
            .reshape(NG_FULL * PB, KSUB, GCOLS)
        )
        tail = wp[NG_FULL * GCOLS :].reshape(TAILC, KSUB, PB).transpose(2, 1, 0)
        in_maps.append(
            {
                "wmain": np.ascontiguousarray(main),
                "wtail": np.ascontiguousarray(tail),
                "xnt": xp,
            }
        )
    return in_maps, lc2_pj


_NC_CACHE = {}


def _get_nc():
    if "nc" not in _NC_CACHE:
        _NC_CACHE["nc"] = build_nc()
    return _NC_CACHE["nc"]


def _install_ntff_hook():
    """The agent image's antenv lacks axon_hooks; shim it so trace=True can
    capture NTFF profiles via the ctypes hook in trn_agent_boot."""
    import sys
    import types

    try:
        from antenv.axon_hooks import get_axon_ntff_profile_hook  # noqa: F401
        return
    except ImportError:
        pass
    mod = types.ModuleType("antenv.axon_hooks")
    _state = {"hook": None}
    mod.set_axon_ntff_profile_hook = lambda h: _state.__setitem__("hook", h)
    mod.get_axon_ntff_profile_hook = lambda: _state["hook"]
    sys.modules["antenv.axon_hooks"] = mod
    import antenv

    antenv.axon_hooks = mod
    from trn_agent_boot.trn_boot import _ntff_profile_via_ctypes

    mod.set_axon_ntff_profile_hook(
        _ntff_profile_via_ctypes("/opt/axon/libaxon_pjrt.so")
    )
    # keep trace artifacts local (no external upload from this sandbox)
    import concourse.bass_utils as bu

    bu.upload_artifacts = lambda tmpdir: tmpdir


def finish_loss(results, lc2_pj):
    """Host epilogue: sum the 8 per-core partials, remove the exact
    zero-pad contribution, log, add label term, mean."""
    Z = np.zeros((PB, BBLK), dtype=np.float64)
    for r in results:
        Z += r["zp"].astype(np.float64)
    # pads: tail-tile cols 212..255 are zero logits on the VectorE
    # Schraudolph path -> each contributes the bit-deterministic image of
    # rint(fp32(SCH_BEFF)) reinterpreted as fp32
    n_pad = CS_PAD - CS                      # 44
    pad_img = np.int32(np.rint(np.float32(0.0) * np.float32(SCH_A)
                               + np.float32(SCH_BEFF)))
    pad_val = float(np.frombuffer(pad_img.tobytes(), dtype=np.float32)[0])
    Z -= NCORES * n_pad * pad_val
    loss = float((np.log(Z) + lc2_pj).mean())
    return np.float32(loss)


def run(x, weight, label, trace=False):
    """Returns (loss_scalar, BassKernelResults)."""
    if trace:
        _install_ntff_hook()
    nc = _get_nc()
    in_maps, lc2_pj = prepare_inputs(x, weight, label)
    res = run_bass_kernel_spmd(
        nc, in_maps, core_ids=list(range(NCORES)), trace=trace
    )
    loss = finish_loss(res.results, lc2_pj)
    return loss, res


def kernel(x, weight, label, batch=None, **_ignored):
    loss, _ = run(x, weight, label, trace=False)
    return np.asarray(loss, dtype=np.float32)


# revision 5
# speedup vs baseline: 1.2873x; 1.2873x over previous
"""ArcFace loss kernel for 8 TRN2 NeuronCores (vocab/tensor-parallel).

reference:
    xn = normalize(x)               # [B, D]
    wn = normalize(weight)          # [C, D]
    logits = 64 * xn @ wn.T         # [B, C]
    loss = mean(CE(logits, label))

Strategy: shard classes C=100000 over 8 cores (12500 each, zero-padded to
12544 = 24*512 + 256). Host prepares normalized fp8(e4m3) operands scaled
by G=8 (so device cosines are 64*cos), pre-packed in the exact SBUF tile
layout so every weight-group DMA is 8KB-contiguous per partition. Each core
computes its logit shard with TensorE fp8 DoubleRow matmuls (K=256/op) into
fp32 PSUM.

The sum-of-exp over each PSUM tile is split across two engines so the PE
stream (8 x 216ns matmuls = ~1.73us/tile) is the only bottleneck:
  - ScalarE: one fused exp(l - SHIFT) activation over a dedicated 3-bank
    PSUM tile (1536 cols) with row-accumulate (~1.54us + 182ns drain).
  - VectorE: Schraudolph exp on a dedicated 1-bank PSUM tile (512 cols):
    i32 = rint(A*l + Beff) via one tensor_scalar (fp32 mul-add, HW
    verified round-to-nearest), whose int32 bits ARE the fp32 exp image;
    a tensor_reduce over the bitcast sums it (~1.4us total). The
    mean-centering constant absorbs the (1+f)2^-f mantissa sawtooth
    (E[g]=1.040684); residual loss error ~1e-4 relative.
  The two consumers read DISJOINT PSUM tiles (separate pools): Tile's
  access tracker serializes cross-engine readers of one tile, which
  otherwise stalls the PE every other tile and trips HAM re-throttling.

DMA: every HWDGE trigger costs ~650ns of sequencer time, serially, so
transfers are packed to minimize trigger count ahead of the critical
path: x and w0 ride one dram tensor (wx0, k-halved so the first matmuls
start ~1.2us earlier), w5 and the tail share another. All on the sync
ring in exact consumption order; one ring already sprays all 16 SDMA
engines at ~420GB/s aggregate.

Tail tiles (256 cols) go entirely to VectorE (ScalarE runs at zero slack);
the 44 zero-pad columns contribute a bit-deterministic Schraudolph image
of exp(-SHIFT) that the host subtracts exactly. Each core returns per-row
partial sums [128, 4]; the host gathers the 8 cores and finishes
loss = mean(log Z + SHIFT - 64*cos_label) with host-exact label cosines.
"""

import math
import numpy as np

import concourse.mybir as mybir
import concourse.tile as tile
from concourse import bacc
from concourse.bass_utils import run_bass_kernel_spmd

# Problem constants (hardcoded per harness contract).
B = 512
D = 512
C = 100000
S = 64.0
SHIFT = 20.0  # logsumexp shift; keeps Z ~1e-2
EPS = 1e-12
G = 8.0      # fp8 pre-scale on both operands: device cos' = G^2 * cos
NCORES = 8
CS = C // NCORES        # true classes per core = 12500
CHUNK = 512             # matmul moving free dim = one full PSUM bank
TAILC = 256             # tail chunk width (212 real + 44 pad cols)
CS_PAD = 24 * CHUNK + TAILC  # padded classes per core = 12544
ASUB = 3                # psum banks per tile consumed by ScalarE
GROUP = 4               # psum banks (512-col chunks) per full tile
NG_FULL = 6             # full groups of 4 chunks; + 1 tail group of 1 chunk
GCOLS = GROUP * CHUNK   # 2048 logit columns per full group
PB = 128                # partitions
KSUB = D // PB          # 4 contraction subtiles of 128
BBLK = B // PB          # 4 batch blocks
NG = NG_FULL + 1        # total groups per core
N_WARM = 20             # PE warm-up matmuls issued while the first DMAs land

ACT_COLS = ASUB * CHUNK       # 1536 cols on ScalarE
XW = B + GCOLS                # wx0 packed width (x | w0)
WR = GCOLS + TAILC            # wrest packed width (w5 | wtail)

# Schraudolph exp-by-bitcast constants (fp32): exp(x) ~= bitcast_f32(
#   rint(A*x + B0)), mean-centered over the mantissa sawtooth.
SCH_A = float(2.0**23 / math.log(2.0))                   # 12102203.1616
SCH_EG = 1.040684490502804                               # E[(1+f)2^-f]
SCH_B0 = (127.0 - math.log(SCH_EG) / math.log(2.0)) * 2.0**23
SCH_BEFF = SCH_B0 - SCH_A * SHIFT                        # folds the shift

F32 = mybir.dt.float32
I32 = mybir.dt.int32
BF16 = mybir.dt.bfloat16
FP8 = mybir.dt.float8e4
NP_FP8 = mybir.dt.np(FP8)


def build_nc(ncores: int = NCORES):
    """Build the SPMD Bass graph."""
    nc = bacc.Bacc(
        "TRN2",
        target_bir_lowering=False,
        debug=False,
        num_devices=ncores,
    )

    # host-packed operands: per-partition-contiguous SBUF layouts
    wx0_ext = nc.dram_tensor("wx0", [PB, KSUB, XW], FP8, kind="ExternalInput")
    wmid_ext = nc.dram_tensor(
        "wmid", [4 * PB, KSUB, GCOLS], FP8, kind="ExternalInput"
    )
    wrest_ext = nc.dram_tensor("wrest", [PB, KSUB, WR], FP8, kind="ExternalInput")
    zp_ext = nc.dram_tensor("zp", [PB, BBLK], F32, kind="ExternalOutput")

    with tile.TileContext(nc) as tc:
        with (
            tc.tile_pool(name="const", bufs=1) as cpool,
            tc.tile_pool(name="wpool", bufs=4) as wpool,
            tc.tile_pool(name="dpool", bufs=3) as dpool,
            tc.tile_pool(name="ipool", bufs=3) as ipool,
        ):
            # ALL loads on the sync HWDGE ring, in consumption order (one
            # ring sprays all 16 SDMA engines; FIFO order == arrival order;
            # each trigger costs ~650ns of sequencer time).
            wx0 = cpool.tile([PB, KSUB, XW], FP8)
            nc.sync.dma_start(out=wx0[:, 0:2, :], in_=wx0_ext[:, 0:2, :])
            nc.sync.dma_start(out=wx0[:, 2:4, :], in_=wx0_ext[:, 2:4, :])
            wmids = []
            for g in range(4):
                wt = wpool.tile([PB, KSUB, GCOLS], FP8, name="wt", tag="w")
                nc.sync.dma_start(out=wt, in_=wmid_ext[g * PB : (g + 1) * PB, :, :])
                wmids.append(wt)
            wrest = cpool.tile([PB, KSUB, WR], FP8)
            nc.sync.dma_start(out=wrest, in_=wrest_ext[:])

            xsb = wx0[:, :, 0:B]
            # per-group weight APs: group 0 lives in wx0, 1-4 in wpool,
            # 5 + tail in wrest
            waps = [wx0[:, :, B : B + GCOLS]]
            waps += [w[:, :, :] for w in wmids]
            waps.append(wrest[:, :, 0:GCOLS])
            waps.append(wrest[:, :, GCOLS : GCOLS + TAILC])

            # warm-up operand first so the PE can start immediately
            warm = cpool.tile([PB, 256], BF16)
            nc.vector.memset(warm, 0.0)

            # constants
            negs = cpool.tile([PB, 1], F32)
            nc.vector.memset(negs, -SHIFT)

            # per-row partial sums, one tile per engine (sharing one tile
            # creates a false cross-engine WAW serialization)
            pSa = cpool.tile([PB, BBLK, NG_FULL], F32)      # ScalarE accum
            pSv = cpool.tile([PB, BBLK, NG], F32)           # VectorE sums

            # preload the Exp activation table off the critical path
            dumdum = cpool.tile([PB, 1], BF16)
            nc.scalar.activation(
                out=dumdum, in_=negs,
                func=mybir.ActivationFunctionType.Exp, bias=negs, scale=1.0,
            )

            with (
                tc.tile_pool(name="psa", bufs=2, space="PSUM") as pool_a,
                tc.tile_pool(name="psv", bufs=2, space="PSUM") as pool_v,
            ):
                # PE warm-up: dependency-free matmuls so the HAM clock gate
                # is released by the time the first weight tiles arrive.
                ones_bf = nc.const_aps.aps[(BF16, 1.0)]
                warm_ps = pool_a.tile(
                    [PB, ASUB, CHUNK], F32, name="warm_ps", tag="psa",
                )
                for _ in range(N_WARM):
                    nc.tensor.matmul(
                        out=warm_ps[0:1, 0, :256], lhsT=ones_bf, rhs=warm,
                        start=True, stop=True,
                    )

                # full groups in DMA-arrival order; tail tiles slotted where
                # VectorE has slack, one kept last so the ending is short
                tiles = [(g, bb) for g in range(3) for bb in range(BBLK)]
                tiles += [(NG - 1, 0)]
                tiles += [(3, bb) for bb in range(BBLK)]
                tiles += [(NG - 1, 1)]
                tiles += [(4, bb) for bb in range(BBLK)]
                tiles += [(NG - 1, 2)]
                tiles += [(5, bb) for bb in range(BBLK)]
                tiles += [(NG - 1, 3)]
                for g, bb in tiles:
                    wt = waps[g]
                    if g < NG_FULL:
                        # ScalarE's 3 banks + VectorE's 1 bank, disjoint
                        # PSUM tiles so the consumers never serialize
                        ps_a = pool_a.tile(
                            [PB, ASUB, CHUNK], F32, name="ps_a", tag="psa",
                        )
                        ps_v = pool_v.tile(
                            [PB, CHUNK], F32, name="ps_v", tag="psv",
                        )
                        for k2 in range(KSUB // 2):
                            for sub in range(GROUP):
                                out_ap = (
                                    ps_a[:, sub : sub + 1, :]
                                    if sub < ASUB
                                    else ps_v[:, 0:CHUNK].unsqueeze(1)
                                )
                                nc.tensor.matmul(
                                    out=out_ap,
                                    lhsT=xsb[
                                        :, 2 * k2 : 2 * k2 + 2,
                                        bb * PB : (bb + 1) * PB,
                                    ],
                                    rhs=wt[
                                        :, 2 * k2 : 2 * k2 + 2,
                                        sub * CHUNK : (sub + 1) * CHUNK,
                                    ],
                                    start=(k2 == 0),
                                    stop=(k2 == KSUB // 2 - 1),
                                    perf_mode=mybir.MatmulPerfMode.DoubleRow,
                                )
                        # ScalarE: exp(l - SHIFT) over the 3-bank tile
                        flat = ps_a.rearrange("p s c -> p (s c)")
                        dump = dpool.tile(
                            [PB, ACT_COLS], BF16, name="dump", tag="dump",
                        )
                        nc.scalar.activation(
                            out=dump,
                            in_=flat,
                            func=mybir.ActivationFunctionType.Exp,
                            bias=negs,
                            scale=1.0,
                            accum_out=pSa[:, bb, g : g + 1],
                        )
                        # VectorE: Schraudolph exp over the 1-bank tile,
                        # summed via the int32->fp32 bitcast
                        idump = ipool.tile(
                            [PB, CHUNK], I32, name="idump", tag="idump",
                        )
                        nc.vector.tensor_scalar(
                            out=idump,
                            in0=ps_v,
                            scalar1=SCH_A,
                            scalar2=SCH_BEFF,
                            op0=mybir.AluOpType.mult,
                            op1=mybir.AluOpType.add,
                        )
                        nc.vector.tensor_reduce(
                            pSv[:, bb, g : g + 1],
                            idump.bitcast(F32),
                            axis=mybir.AxisListType.X,
                            op=mybir.AluOpType.add,
                        )
                    else:
                        # tail tile: one bank, entirely VectorE
                        ps_v = pool_v.tile(
                            [PB, CHUNK], F32, name="ps_vt", tag="psv",
                        )
                        for k2 in range(KSUB // 2):
                            nc.tensor.matmul(
                                out=ps_v[:, 0:TAILC].unsqueeze(1),
                                lhsT=xsb[
                                    :, 2 * k2 : 2 * k2 + 2,
                                    bb * PB : (bb + 1) * PB,
                                ],
                                rhs=wt[:, 2 * k2 : 2 * k2 + 2, 0:TAILC],
                                start=(k2 == 0),
                                stop=(k2 == KSUB // 2 - 1),
                                perf_mode=mybir.MatmulPerfMode.DoubleRow,
                            )
                        idump = ipool.tile(
                            [PB, TAILC], I32, name="idumpt", tag="idump",
                            padded_shape=[PB, CHUNK],
                        )
                        nc.vector.tensor_scalar(
                            out=idump,
                            in0=ps_v[:, 0:TAILC],
                            scalar1=SCH_A,
                            scalar2=SCH_BEFF,
                            op0=mybir.AluOpType.mult,
                            op1=mybir.AluOpType.add,
                        )
                        nc.vector.tensor_reduce(
                            pSv[:, bb, NG_FULL : NG_FULL + 1],
                            idump.bitcast(F32),
                            axis=mybir.AxisListType.X,
                            op=mybir.AluOpType.add,
                        )

            # Z partial per row: [128, BBLK] -> output (host sums the 8 cores)
            zpa = cpool.tile([PB, BBLK], F32)
            nc.vector.tensor_reduce(
                zpa, pSa, axis=mybir.AxisListType.X, op=mybir.AluOpType.add,
            )
            zpv = cpool.tile([PB, BBLK], F32)
            nc.vector.tensor_reduce(
                zpv, pSv, axis=mybir.AxisListType.X, op=mybir.AluOpType.add,
            )
            zp = cpool.tile([PB, BBLK], F32)
            nc.vector.tensor_add(out=zp, in0=zpa, in1=zpv)
            nc.sync.dma_start(out=zp_ext[:], in_=zp)

    nc.finalize()
    return nc


def prepare_inputs(x, weight, label, ncores: int = NCORES):
    """Host-side prep: normalize, G-scale, cast fp8, pack to SBUF layouts.

    Returns (in_maps, lc2) where lc2[p, j] = SHIFT - S*cos(x_b, w_label_b)
    for b = j*128 + p."""
    x = np.asarray(x, dtype=np.float32)
    weight = np.asarray(weight, dtype=np.float32)
    label = np.asarray(label).astype(np.int64)

    xn = x / np.maximum(
        np.sqrt(np.einsum("bd,bd->b", x, x, dtype=np.float64))[:, None], EPS
    ).astype(np.float32)
    wnorm = np.sqrt(np.einsum("cd,cd->c", weight, weight, dtype=np.float64))
    wn = weight / np.maximum(wnorm[:, None], EPS).astype(np.float32)

    # label cosine computed on host in f64 (exact vs fp32 reference)
    wl = wn[label]  # [B, D]
    label_cos = np.einsum("bd,bd->b", xn.astype(np.float64), wl.astype(np.float64))
    lc2 = (SHIFT - S * label_cos).astype(np.float64)  # [B]
    lc2_pj = np.ascontiguousarray(lc2.reshape(BBLK, PB).T)  # [128, BBLK]

    x8 = (G * xn).astype(NP_FP8)          # [B, D]
    w8 = (G * wn).astype(NP_FP8)          # [C, D]
    # xnt[p, ks, b] = x8[b, ks*128 + p]
    xp = x8.reshape(B, KSUB, PB).transpose(2, 1, 0)  # [128, 4, 512]

    in_maps = []
    for i in range(ncores):
        wp = np.zeros((CS_PAD, D), dtype=NP_FP8)
        wp[:CS] = w8[i * CS : (i + 1) * CS]
        # group g block: [p, ks, col] = wp[g*2048 + col, ks*128 + p]
        main = (
            wp[: NG_FULL * GCOLS]
            .reshape(NG_FULL, GCOLS, KSUB, PB)
            .transpose(0, 3, 2, 1)  # [6, 128, 4, 2048]
        )
        tail = wp[NG_FULL * GCOLS :].reshape(TAILC, KSUB, PB).transpose(2, 1, 0)
        wx0 = np.concatenate([xp, main[0]], axis=2)          # [128, 4, 2560]
        wmid = main[1:5].reshape(4 * PB, KSUB, GCOLS)
        wrest = np.concatenate([main[5], tail], axis=2)      # [128, 4, 2304]
        in_maps.append(
            {
                "wx0": np.ascontiguousarray(wx0),
                "wmid": np.ascontiguousarray(wmid),
                "wrest": np.ascontiguousarray(wrest),
            }
        )
    return in_maps, lc2_pj


_NC_CACHE = {}


def _get_nc():
    if "nc" not in _NC_CACHE:
        _NC_CACHE["nc"] = build_nc()
    return _NC_CACHE["nc"]


def _install_ntff_hook():
    """The agent image's antenv lacks axon_hooks; shim it so trace=True can
    capture NTFF profiles via the ctypes hook in trn_agent_boot."""
    import sys
    import types

    try:
        from antenv.axon_hooks import get_axon_ntff_profile_hook  # noqa: F401
        return
    except ImportError:
        pass
    mod = types.ModuleType("antenv.axon_hooks")
    _state = {"hook": None}
    mod.set_axon_ntff_profile_hook = lambda h: _state.__setitem__("hook", h)
    mod.get_axon_ntff_profile_hook = lambda: _state["hook"]
    sys.modules["antenv.axon_hooks"] = mod
    import antenv

    antenv.axon_hooks = mod
    from trn_agent_boot.trn_boot import _ntff_profile_via_ctypes

    mod.set_axon_ntff_profile_hook(
        _ntff_profile_via_ctypes("/opt/axon/libaxon_pjrt.so")
    )
    # keep trace artifacts local (no external upload from this sandbox)
    import concourse.bass_utils as bu

    bu.upload_artifacts = lambda tmpdir: tmpdir


def finish_loss(results, lc2_pj):
    """Host epilogue: sum the 8 per-core partials, remove the exact
    zero-pad contribution, log, add label term, mean."""
    Z = np.zeros((PB, BBLK), dtype=np.float64)
    for r in results:
        Z += r["zp"].astype(np.float64)
    # pads: tail-tile cols 212..255 are zero logits on the VectorE
    # Schraudolph path -> each contributes the bit-deterministic image of
    # rint(fp32(SCH_BEFF)) reinterpreted as fp32
    n_pad = CS_PAD - CS                      # 44
    pad_img = np.int32(np.rint(np.float32(0.0) * np.float32(SCH_A)
                               + np.float32(SCH_BEFF)))
    pad_val = float(np.frombuffer(pad_img.tobytes(), dtype=np.float32)[0])
    Z -= NCORES * n_pad * pad_val
    loss = float((np.log(Z) + lc2_pj).mean())
    return np.float32(loss)


def run(x, weight, label, trace=False):
    """Returns (loss_scalar, BassKernelResults)."""
    if trace:
        _install_ntff_hook()
    nc = _get_nc()
    in_maps, lc2_pj = prepare_inputs(x, weight, label)
    res = run_bass_kernel_spmd(
        nc, in_maps, core_ids=list(range(NCORES)), trace=trace
    )
    loss = finish_loss(res.results, lc2_pj)
    return loss, res


def kernel(x, weight, label, batch=None, **_ignored):
    loss, _ = run(x, weight, label, trace=False)
    return np.asarray(loss, dtype=np.float32)
